# revision 32
# baseline (speedup 1.0000x reference)
"""GCMC message-passing kernel for trn2, v2: wire-byte-minimized.

Per core = one dst-shard of nodes, both directions (0: drug->dis, 1: dis->drug).
  Phase W: W[r] = att @ basis -> 5 wtab_r tensors [IN,128] bf16 (rows padded
    to 256B for dma_gather).
  Phase X: per (d,k): gather wtab_r rows by node feature -> X tables
    XA[d,k,r] (nodes 0:32768) / XB[d,k,r] (nodes 32768:50000, padded to
    32768 rows, pad zero-filled), rows scaled by cj[node].  Same X on every
    core; gather idx wire is the packed feature column (shared).
  Phase E: per (d,k): per-edge events sorted by slot (r-major, dst-local),
    within-slot split A (src<32768) then B.  One u16 src wire stream per
    direction; device computes idx = src & 0x7FFF; each 128-event window is
    gathered twice (XA and XB tables) and masked by two one-hot SegT
    matrices built from per-slot boundary step functions
    (G >= startA) - (G >= startB) etc., so no per-edge slot/scale bytes.
    PE: msgs.T @ SegT accumulated into psum pages [MU,128] -> hT bf16 HBM.
  Phase P: outT[d] = sum_rk fcblk.T @ hT-slices, x ci (DMA-broadcast cols),
    + bias, then int8-quantized per (d,half,chunk,row) with scales in a
    separate small output (host dequantizes).

Wire per core: src u16 [16,NCALL*64] x2, bnd i16 [1,NPAGE*257] x2,
  xidx i16 [16,XCALL*64] x6, cj f32 [128,392] x2, ci f32 [1,SH] x2,
  basisf/fcrT bf16, G/fcb f32; out: outT i8 [2,OUT,SH] + oscale f32.
"""
import numpy as np
import ml_dtypes
import concourse.bass as bass
import concourse.bacc as bacc
import concourse.mybir as mybir

F32 = mybir.dt.float32
BF16 = mybir.dt.bfloat16
I16 = mybir.dt.int16
I8 = mybir.dt.int8

NP_BF16 = ml_dtypes.bfloat16

R = 5
MU = 64
OUT = 256
NK = 3
IN = 1024
HALF = 32768


class Cfg:
    def __init__(self, n_nodes, in_units, n_cores, wpp):
        self.N = n_nodes
        self.IN = in_units
        self.NC = n_cores
        self.SH = n_nodes // n_cores          # 6250
        self.PPR = (self.SH + 127) // 128     # 49
        self.NPAGE = R * self.PPR             # 245
        self.NSLOT = self.NPAGE * 128         # 31360
        self.WPP = wpp                        # 12
        self.WLAST = wpp + 4                  # 16 (r-block pad windows)
        self.NWR = self.PPR * wpp + 4         # 592 (mult of 8)
        assert self.NWR % 8 == 0
        self.NWP = R * self.NWR               # 2960
        self.NCALL = self.NWP // 8            # 370
        self.CPR = self.NWR // 8              # 74 calls per rating
        self.CH = 37                          # calls per idx chunk
        self.NCHK = self.NCALL // self.CH     # 10
        assert self.NCALL % self.CH == 0
        self.DCH = 512
        self.NDC = (self.SH + self.DCH - 1) // self.DCH  # 13
        self.SPS = 16
        self.NSTG = (self.NPAGE + self.SPS - 1) // self.SPS  # 16
        self.WROUND = (in_units * MU) // 2048  # 32
        self.XCALL = (n_nodes + 1023) // 1024  # 49
        self.XNW = (n_nodes + 127) // 128      # 391
        self.XNWP = 392                        # padded cj windows
        self.NB = n_nodes - HALF               # 17232 real B rows

    # page tables (per pass): page p -> (rating, first window, nwin)
    def page_geom(self, p):
        r, pl = divmod(p, self.PPR)
        w0 = r * self.NWR + pl * self.WPP
        nwin = self.WPP if pl < self.PPR - 1 else self.WLAST
        return r, w0, nwin


def _pack_calls(evt, ncall):
    """evt [ncall*1024] -> [16, ncall*64] in dma_gather order."""
    return evt.reshape(ncall, 64, 16).transpose(2, 0, 1).reshape(16, ncall * 64)


def host_prep_dir(cfg, feat_src, cj_src, src, dst, lo):
    """Edge streams for one direction on one core (dst-shard [lo, lo+SH))."""
    SH, PPR, WPP, NWR = cfg.SH, cfg.PPR, cfg.WPP, cfg.NWR
    slots, srcs = [], []
    for r in range(R):
        m = (dst[r] >= lo) & (dst[r] < lo + SH)
        s, d = src[r][m], dst[r][m]
        slots.append(r * PPR * 128 + (d - lo))
        srcs.append(s)
    slot = np.concatenate(slots).astype(np.int64)
    srcv = np.concatenate(srcs).astype(np.int64)
    key = slot * 2 + (srcv >= HALF)
    order = np.argsort(key, kind="stable")
    slot, srcv, key = slot[order], srcv[order], key[order]

    cnt = np.bincount(key, minlength=2 * cfg.NSLOT)
    cum = np.concatenate([[0], np.cumsum(cnt)])  # exclusive prefix, len 2*NSLOT+1
    bnd = np.zeros((1, cfg.NPAGE * 257), np.int16)
    pgcnt = np.zeros(cfg.NPAGE, np.int64)
    for p in range(cfg.NPAGE):
        base = cum[2 * 128 * p]
        n_p = cum[2 * 128 * (p + 1)] - base
        assert n_p <= cfg.WLAST * 128 if p % PPR == PPR - 1 else n_p <= WPP * 128, \
            f"page {p} overflow: {n_p}"
        bnd[0, p * 257: p * 257 + 256] = cum[2 * 128 * p: 2 * 128 * (p + 1)] - base
        bnd[0, p * 257 + 256] = n_p
        pgcnt[p] = n_p

    # event stream positions: page-contiguous, page p starts at window w0(p)
    evt = np.zeros(cfg.NWP * 128, np.uint16)
    page = slot // 128
    pagestart = np.concatenate([[0], np.cumsum(pgcnt)])[:-1]
    rank = np.arange(len(slot)) - pagestart[page]
    r_of_p, pl_of_p = page // PPR, page % PPR
    w0 = r_of_p * NWR + pl_of_p * WPP
    pos = w0 * 128 + rank
    evt[pos] = srcv.astype(np.uint16)
    g = _pack_calls(evt, cfg.NCALL).astype(np.uint16).view(np.int16).copy()
    return g, bnd


def build_inputs(cfg, inputs):
    f32 = np.float32
    gi = lambda n: np.asarray(inputs[n], np.int64)
    gf = lambda n: np.asarray(inputs[n], f32)
    drug_feat, dis_feat = gi("drug_feat"), gi("dis_feat")
    src, dst = gi("src"), gi("dst")
    cj_drug, ci_drug = gf("cj_drug"), gf("ci_drug")
    cj_dis, ci_dis = gf("cj_dis"), gf("ci_dis")
    att, basis = gf("att"), gf("basis")
    fc_w, fc_b = gf("fc_w"), gf("fc_b")

    attT = att.T.astype(NP_BF16).copy()
    basisf = basis.reshape(4, cfg.IN * MU).astype(NP_BF16).copy()
    fcrT = fc_w.reshape(R * NK, MU, OUT).transpose(1, 0, 2).astype(NP_BF16).copy()
    fcb2 = fc_b.reshape(2, 128).T.copy().astype(f32)
    Gtab = (np.arange(cfg.WLAST)[None, :] * 128
            + np.arange(128)[:, None]).astype(f32)

    shared = {"attT": attT, "basisf": basisf, "fcrT": fcrT, "fcb2": fcb2,
              "gt": Gtab}
    feats = (drug_feat, dis_feat)
    cjs = (cj_drug, cj_dis)
    cis = (ci_dis, ci_drug)   # d=0 output is dis (scaled by ci_dis)
    for d in range(2):
        for k in range(NK):
            evtX = np.zeros(cfg.XCALL * 1024, np.int64)
            evtX[:cfg.N] = feats[d][:, k]
            shared[f"xg{d}{k}"] = _pack_calls(evtX, cfg.XCALL).astype(np.int16)
        cjt = np.zeros(cfg.XNWP * 128, f32)
        cjt[:cfg.N] = cjs[d][:, 0]
        shared[f"cj{d}"] = cjt.reshape(cfg.XNWP, 128).T.copy()

    maps = []
    for core in range(cfg.NC):
        lo = core * cfg.SH
        m = dict(shared)
        for d, (fs, cj, sr, ds) in enumerate(
                [(drug_feat, cj_drug, src, dst), (dis_feat, cj_dis, dst, src)]):
            g, bnd = host_prep_dir(cfg, fs, cj, sr, ds, lo)
            m[f"g{d}"], m[f"bnd{d}"] = g, bnd
            m[f"ci{d}"] = cis[d][lo:lo + cfg.SH].reshape(1, cfg.SH).copy()
        maps.append(m)
    return maps


def assemble_output(cfg, results):
    NDC, DCH, SH = cfg.NDC, cfg.DCH, cfg.SH
    outs = []
    for d in range(2):
        parts = []
        for c in range(cfg.NC):
            q = results[c]["outT"][d].astype(np.float32)   # [256, SH]
            sc = results[c]["oscale"] / 127.0              # [128, 2*NDC*2] raw absmax
            for cc in range(NDC):
                c0, c1 = cc * DCH, min((cc + 1) * DCH, SH)
                for h in range(2):
                    q[h * 128:(h + 1) * 128, c0:c1] *= \
                        sc[:, (d * NDC + cc) * 2 + h][:, None]
            parts.append(q.T)
        outs.append(np.concatenate(parts, 0))
    dis_out, drug_out = outs
    return drug_out, dis_out


def build_kernel(cfg, debug=True, taps=False):
    nc = bacc.Bacc(None, target_bir_lowering=False, debug=debug)
    NCALL, NWP, NPAGE, WPP, WLAST = cfg.NCALL, cfg.NWP, cfg.NPAGE, cfg.WPP, cfg.WLAST
    SH, PPR, NSLOT, DCH, NDC = cfg.SH, cfg.PPR, cfg.NSLOT, cfg.DCH, cfg.NDC
    SPS, NSTG, WROUND = cfg.SPS, cfg.NSTG, cfg.WROUND
    CH, NCHK, CPR = cfg.CH, cfg.NCHK, cfg.CPR
    XCALL, NB = cfg.XCALL, cfg.NB
    PASSES = [(d, k) for d in range(2) for k in range(NK)]

    # ---- DRAM params
    attT_d = nc.declare_dram_parameter("attT", [4, R], BF16, isOutput=False)
    basisf_d = nc.declare_dram_parameter("basisf", [4, cfg.IN * MU], BF16, isOutput=False)
    fcr_d = nc.declare_dram_parameter("fcrT", [MU, R * NK, OUT], BF16, isOutput=False)
    fcb_d = nc.declare_dram_parameter("fcb2", [128, 2], F32, isOutput=False)
    gt_d = nc.declare_dram_parameter("gt", [128, WLAST], F32, isOutput=False)
    xg_d, cj_d, ci_d, gD, bndD = {}, {}, {}, {}, {}
    for d in range(2):
        for k in range(NK):
            xg_d[d, k] = nc.declare_dram_parameter(f"xg{d}{k}", [16, XCALL * 64], I16, isOutput=False)
        cj_d[d] = nc.declare_dram_parameter(f"cj{d}", [128, cfg.XNWP], F32, isOutput=False)
        ci_d[d] = nc.declare_dram_parameter(f"ci{d}", [1, SH], F32, isOutput=False)
        gD[d] = nc.declare_dram_parameter(f"g{d}", [16, NCALL * 64], I16, isOutput=False)
        bndD[d] = nc.declare_dram_parameter(f"bnd{d}", [1, NPAGE * 257], I16, isOutput=False)
    outT_d = nc.declare_dram_parameter("outT", [2, OUT, SH], I8, isOutput=True)
    osc_d = nc.declare_dram_parameter("oscale", [128, 2 * NDC * 2], F32, isOutput=True)
    if taps:
        tap_d = nc.declare_dram_parameter("tap", [128, 4 * DCH + 16], F32, isOutput=True)

    # ---- DRAM scratch (taps=True turns some into outputs for debugging)
    if taps:
        dscratch = lambda name, shape, dt: nc.declare_dram_parameter(
            name, shape, dt, isOutput=True)
    else:
        dscratch = nc.dram_tensor
    wtab = [nc.dram_tensor(f"wtab{r}", [1, cfg.IN * 128], BF16) for r in range(R)]
    wrows = [wtab[r][:].rearrange("o (f m) -> (o f) m", m=128) for r in range(R)]
    XA = {(d, k): [(dscratch if (d, k, r) == (0, 0, 0) else nc.dram_tensor)(
        f"xa{d}{k}{r}", [HALF, 128], BF16) for r in range(R)] for d, k in PASSES}
    XB = {(d, k): [(dscratch if (d, k, r) == (0, 0, 0) else nc.dram_tensor)(
        f"xb{d}{k}{r}", [HALF, 128], BF16) for r in range(R)] for d, k in PASSES}
    hT = dscratch("hT", [2, NK, MU, NSLOT], BF16)

    # ---- SBUF
    attT_sb = nc.alloc_sbuf_tensor("attT_sb", [4, R], BF16)
    bchunk = nc.alloc_sbuf_tensor("bchunk", [4, 2048], BF16)
    wstage = nc.alloc_sbuf_tensor("wstage", [R, 4096], BF16)
    gt_sb = nc.alloc_sbuf_tensor("gt_sb", [128, WLAST], F32)
    fcr_sb = nc.alloc_sbuf_tensor("fcr_sb", [MU, R * NK, OUT], BF16)
    fcb_sb = nc.alloc_sbuf_tensor("fcb_sb", [128, 2], F32)
    zfill = nc.alloc_sbuf_tensor("zfill", [128, 16, 128], BF16)
    xgsb = nc.alloc_sbuf_tensor("xgsb", [128, XCALL * 64], I16)
    cjsb = nc.alloc_sbuf_tensor("cjsb", [128, cfg.XNWP], F32)
    xstage = [nc.alloc_sbuf_tensor(f"xstage{i}", [128, 8, 64], BF16) for i in range(4)]
    NMB = 8
    mA = [nc.alloc_sbuf_tensor(f"mA{i}", [128, 8, 128], BF16) for i in range(NMB)]
    mB = [nc.alloc_sbuf_tensor(f"mB{i}", [128, 8, 128], BF16) for i in range(NMB)]
    graw = [nc.alloc_sbuf_tensor(f"graw{i}", [128, CH * 64], I16) for i in range(2)]
    gsbA = [nc.alloc_sbuf_tensor(f"gsbA{i}", [128, CH * 64], I16) for i in range(2)]
    NSB = 3
    bndi = [nc.alloc_sbuf_tensor(f"bndi{i}", [128, 260], I16) for i in range(NSB)]
    bndf = [nc.alloc_sbuf_tensor(f"bndf{i}", [128, 260], F32) for i in range(NSB)]
    segA = [nc.alloc_sbuf_tensor(f"segA{i}", [128, WLAST, 128], BF16) for i in range(NSB)]
    segB = [nc.alloc_sbuf_tensor(f"segB{i}", [128, WLAST, 128], BF16) for i in range(NSB)]
    tmpu = nc.alloc_sbuf_tensor("tmpu", [128, WLAST, 128], BF16)
    tmpu2 = nc.alloc_sbuf_tensor("tmpu2", [128, WLAST, 128], BF16)
    NSTB = 2
    stage = [nc.alloc_sbuf_tensor(f"stage{i}", [MU, SPS * 128], BF16) for i in range(NSTB)]
    prhs = [nc.alloc_sbuf_tensor(f"prhs{i}", [MU, R * NK, DCH], BF16) for i in range(2)]
    cirep = [nc.alloc_sbuf_tensor(f"cirep{i}", [128, DCH], F32) for i in range(2)]
    tmpq = [nc.alloc_sbuf_tensor(f"tmpq{i}", [128, DCH + 4], F32) for i in range(2)]
    qsb = [nc.alloc_sbuf_tensor(f"qsb{i}", [128, DCH], F32) for i in range(2)]
    rcb = nc.alloc_sbuf_tensor("rcb", [128, 64], F32)
    rcb2 = nc.alloc_sbuf_tensor("rcb2", [128, 64], F32)
    onesq = nc.alloc_sbuf_tensor("onesq", [128, 64], F32)
    ostage = [nc.alloc_sbuf_tensor(f"ostage{i}", [128, DCH], I8) for i in range(4)]
    osc_sb = nc.alloc_sbuf_tensor("osc_sb", [128, 2 * NDC * 2], F32)
    if taps:
        tap_sb = nc.alloc_sbuf_tensor("tap_sb", [128, 4 * DCH + 16], F32)

    NPB = 4
    psA = nc.alloc_psum_tensor("psA", [128, 2048], F32)
    pages = [psA[0:MU, i * 512:i * 512 + 128] for i in range(NPB)]
    psB = nc.alloc_psum_tensor("psB", [128, 2048], F32)
    wps = psB[0:R, :]
    pps = [psB[:, j * 512:(j + 1) * 512] for j in range(4)]

    # ---- python-side schedule tables
    geom = [cfg.page_geom(p) for p in range(NPAGE)]       # (r, w0, nwin)
    win2page = np.zeros(NWP, np.int64)
    for p, (r, w0, nwin) in enumerate(geom):
        win2page[w0:w0 + nwin] = p
    mm_cum = np.zeros(NPAGE + 1, np.int64)                # matmuls through page p
    for p, (r, w0, nwin) in enumerate(geom):
        mm_cum[p + 1] = 2 * (w0 + nwin)
    MM_PASS = 2 * NWP                                     # matmuls per pass
    NXC = 6 * R * XCALL                                   # X gather calls total
    # X write DMAs per call (call 48 partial -> 2)
    xw_per_call = [2 if c == XCALL - 1 else 1 for c in range(XCALL)]
    XW_PER_R = sum(xw_per_call)                           # 50
    ZF_DMAS = len(PASSES) * R * 16                        # zero-fill chunks

    sems = {}
    with nc.Block() as block:
        for name, n in [("gthA", NMB), ("gthB", NMB), ("stg", NSTB),
                        ("pin", 2), ("ost", 4)]:
            for i in range(n):
                sems[name, i] = nc.alloc_semaphore(f"s_{name}{i}")
        for name in ["wdma", "wmm", "wcp", "wout", "zini", "zf", "xgl", "xcj",
                     "xst", "xout", "graw", "gab", "bndl", "bcv", "seg", "pe",
                     "act", "pmm", "cir", "oact", "osc", "qmx", "qwd"]:
            sems[name] = nc.alloc_semaphore(f"s_{name}")
        s_gthA = [sems["gthA", i] for i in range(NMB)]
        s_gthB = [sems["gthB", i] for i in range(NMB)]
        s_stg = [sems["stg", i] for i in range(NSTB)]
        s_pin = [sems["pin", i] for i in range(2)]
        s_ost = [sems["ost", i] for i in range(4)]
        # ======== GPSIMD: consts, W-build DMAs, X gathers, edge gathers
        @block.gpsimd
        def _(g):
            for t, dd in [(attT_sb, attT_d), (fcb_sb, fcb_d),
                          (fcr_sb, fcr_d), (gt_sb, gt_d)]:
                g.dma_start(t[:], dd[:]).then_inc(sems["wdma"], 16)
            for n in range(WROUND):
                g.wait_ge(sems["wcp"], n)
                g.dma_start(bchunk[:], basisf_d[:, n * 2048:(n + 1) * 2048]
                            ).then_inc(sems["wdma"], 16)
                g.wait_ge(sems["wcp"], n + 1)
                for r in range(R):
                    g.dma_start(
                        wtab[r][:, n * 4096:(n + 1) * 4096],
                        wstage[r:r + 1, :]).then_inc(sems["wout"], 16)
            g.wait_ge(sems["wout"], WROUND * R * 16)
            # X gathers
            useA = [0] * NMB
            lastA = [None] * NMB
            gu = 0
            for pi, (d, k) in enumerate(PASSES):
                g.wait_ge(sems["xgl"], 128 * (pi + 1))
                for r in range(R):
                    for c in range(XCALL):
                        b = gu % NMB
                        if lastA[b] is not None:
                            g.wait_ge(*lastA[b])
                        g.dma_gather(mA[b][:], wrows[r],
                                     xgsb[:, c * 64:(c + 1) * 64],
                                     1024, 1024, 128).then_inc(s_gthA[b], 16)
                        useA[b] += 1
                        lastA[b] = (sems["xst"], gu + 1)
                        gu += 1
            # edge gathers (two per call: XA and XB halves, same indices)
            lastB = [None] * NMB
            for pi, (d, k) in enumerate(PASSES):
                for c in range(NCALL):
                    b = (pi * NCALL + c) % NMB
                    r = c // CPR
                    ch = c // CH
                    idx_ap = gsbA[ch % 2][:, (c - ch * CH) * 64:
                                          (c - ch * CH + 1) * 64]
                    g.wait_ge(sems["gab"], pi * NCHK + ch + 1)
                    if lastA[b] is not None:
                        g.wait_ge(*lastA[b])
                    g.dma_gather(mA[b][:], XA[d, k][r][:], idx_ap,
                                 1024, 1024, 128).then_inc(s_gthA[b], 16)
                    if lastB[b] is not None:
                        g.wait_ge(*lastB[b])
                    g.dma_gather(mB[b][:], XB[d, k][r][:], idx_ap,
                                 1024, 1024, 128).then_inc(s_gthB[b], 16)
                    pe_after = pi * MM_PASS + 16 * (c + 1)
                    lastA[b] = (sems["pe"], pe_after)
                    lastB[b] = (sems["pe"], pe_after)

        # ======== TENSOR: W mms, edge mms, projection mms
        @block.tensor
        def _(t):
            for n in range(WROUND):
                t.wait_ge(sems["wdma"], 64 + 16 * (n + 1))
                if n >= 1:
                    t.wait_ge(sems["wcp"], n)
                for i in range(4):
                    ins = t.matmul(wps[:, i * 512:(i + 1) * 512], attT_sb[:],
                                   bchunk[:, i * 512:(i + 1) * 512],
                                   start=True, stop=True)
                ins.then_inc(sems["wmm"], 1)
            uA = [0] * NMB
            uB = [0] * NMB
            # replay gather-use counts from the X phase
            for gu in range(NXC):
                uA[gu % NMB] += 1
            for pi, (d, k) in enumerate(PASSES):
                for w in range(NWP):
                    p = int(win2page[w])
                    r, w0, nwin = geom[p]
                    pglob = pi * NPAGE + p
                    first = (w == w0)
                    if first and pglob >= NPB:
                        t.wait_ge(sems["act"], pglob - NPB + 1)
                    if first:
                        t.wait_ge(sems["seg"], pglob + 1)
                    b = ((pi * NCALL) + w // 8) % NMB
                    if w % 8 == 0:
                        uA[b] += 1
                        uB[b] += 1
                        t.wait_ge(s_gthA[b], 16 * uA[b])
                        t.wait_ge(s_gthB[b], 16 * uB[b])
                    last = (w == w0 + nwin - 1)
                    t.matmul(pages[pglob % NPB], mA[b][:, w % 8, 0:MU],
                             segA[pglob % NSB][:, w - w0, :],
                             start=first, stop=False).then_inc(sems["pe"], 1)
                    t.matmul(pages[pglob % NPB], mB[b][:, w % 8, 0:MU],
                             segB[pglob % NSB][:, w - w0, :],
                             start=False, stop=last).then_inc(sems["pe"], 1)
            nl = 0
            for d in range(2):
                for c in range(NDC):
                    ncols = min(DCH, SH - c * DCH)
                    t.wait_ge(s_pin[nl % 2], 240 * (nl // 2 + 1))
                    if nl >= 2:
                        t.wait_ge(sems["oact"], 2 * (nl - 1))
                    for h in range(2):
                        for rk in range(R * NK):
                            ins = t.matmul(pps[2 * (nl % 2) + h][:, :ncols],
                                           fcr_sb[:, rk, h * 128:(h + 1) * 128],
                                           prhs[nl % 2][:, rk, :ncols],
                                           start=(rk == 0), stop=(rk == R * NK - 1))
                        ins.then_inc(sems["pmm"], 1)
                    nl += 1

        # ======== VECTOR: W copies, X scaling, idx AND, SegT, quant
        @block.vector
        def _(v):
            v.memset(zfill[:], 0)
            v.memset(wstage[:], 0)
            for i in range(2):
                v.memset(tmpq[i][:, 0:1], 1e-12)
            v.memset(osc_sb[:], 0)
            vi = v.memset(onesq[:], 1.0 / 127.0)
            vi.then_inc(sems["zini"], 1)
            for n in range(WROUND):
                v.wait_ge(sems["wmm"], n + 1)
                if n >= 1:
                    v.wait_ge(sems["wout"], R * 16 * n)
                v.tensor_copy(
                    wstage[:].rearrange("r (f m) -> r f m", m=128)[:, :, 0:MU],
                    wps[:].rearrange("r (f m) -> r f m", m=MU),
                ).then_inc(sems["wcp"], 1)
            # X scaling: xstage = msgs[:, :, 0:64] * cj
            uA = [0] * NMB
            gu = 0
            for pi, (d, k) in enumerate(PASSES):
                v.wait_ge(sems["xcj"], 16 * (d + 1))
                for r in range(R):
                    for c in range(XCALL):
                        b = gu % NMB
                        uA[b] += 1
                        v.wait_ge(s_gthA[b], 16 * uA[b])
                        if gu >= 4:
                            v.wait_ge(sems["xout"], xout_cum(gu - 3))
                        cj_b = cjsb[:, c * 8:(c + 1) * 8].unsqueeze(2) \
                            .to_broadcast([128, 8, 64])
                        v.scalar_tensor_tensor(
                            xstage[gu % 4][:], mA[b][:, :, 0:MU], 0.0, cj_b,
                            mybir.AluOpType.bypass, mybir.AluOpType.mult,
                        ).then_inc(sems["xst"], 1)
                        gu += 1
            # edge passes: idx AND + SegT builds (lazily interleaved)
            v.wait_ge(sems["wdma"], 64)
            pseq = 0
            for pi, (d, k) in enumerate(PASSES):
                next_ch = 0

                def emit_and(ch, pi=pi):
                    v.wait_ge(sems["graw"], 128 * (pi * NCHK + ch + 1))
                    if pi * NCHK + ch >= 2:
                        v.wait_ge(sems["pe"], pe_done_for_chunk(pi, ch))
                    v.tensor_scalar(gsbA[ch % 2][:], graw[ch % 2][:], 0x7FFF,
                                    None, mybir.AluOpType.bitwise_and
                                    ).then_inc(sems["gab"], 1)

                def pe_done_for_chunk(pi, ch):
                    # gathers of chunk ch-2 are complete once matmuls of its
                    # last window have run
                    gc = pi * NCHK + ch - 2
                    gpi, gch = divmod(gc, NCHK)
                    return gpi * MM_PASS + 16 * CH * (gch + 1)

                for p in range(NPAGE):
                    r, w0, nwin = geom[p]
                    ch_need = min((w0 + nwin) // (8 * CH) + 1, NCHK)
                    while next_ch < ch_need:
                        emit_and(next_ch)
                        next_ch += 1
                    ps = pseq % NSB
                    v.wait_ge(sems["bndl"], 16 * (pseq + 1))
                    if pseq >= NSB:
                        pp = pseq - NSB
                        ppi, ppg = divmod(pp, NPAGE)
                        v.wait_ge(sems["pe"], ppi * MM_PASS + int(mm_cum[ppg + 1]))
                    v.tensor_copy(bndf[ps][:, 0:257], bndi[ps][:, 0:257]
                                  ).then_inc(sems["bcv"], 1)
                    sA = bndf[ps][:, 0:256].rearrange("p (s two) -> p two s", two=2)[:, 0, :]
                    sB = bndf[ps][:, 0:256].rearrange("p (s two) -> p two s", two=2)[:, 1, :]
                    eB = bndf[ps][:, 2:258].rearrange("p (s two) -> p two s", two=2)[:, 0, :]
                    g_b = gt_sb[:, 0:nwin].unsqueeze(2).to_broadcast([128, nwin, 128])
                    sA_b = sA.unsqueeze(1).to_broadcast([128, nwin, 128])
                    sB_b = sB.unsqueeze(1).to_broadcast([128, nwin, 128])
                    eB_b = eB.unsqueeze(1).to_broadcast([128, nwin, 128])
                    vA = segA[ps][:, 0:nwin, :]
                    vB = segB[ps][:, 0:nwin, :]
                    tU = tmpu[:, 0:nwin, :]
                    tU2 = tmpu2[:, 0:nwin, :]
                    v.scalar_tensor_tensor(tU, g_b, 0.0, sB_b,
                                           mybir.AluOpType.bypass, mybir.AluOpType.is_ge)
                    v.scalar_tensor_tensor(vA, g_b, 0.0, sA_b,
                                           mybir.AluOpType.bypass, mybir.AluOpType.is_ge)
                    v.scalar_tensor_tensor(vA, vA, 0.0, tU,
                                           mybir.AluOpType.bypass, mybir.AluOpType.subtract)
                    v.scalar_tensor_tensor(tU2, g_b, 0.0, eB_b,
                                           mybir.AluOpType.bypass, mybir.AluOpType.is_ge)
                    v.scalar_tensor_tensor(vB, tU, 0.0, tU2,
                                           mybir.AluOpType.bypass, mybir.AluOpType.subtract
                                           ).then_inc(sems["seg"], 1)
                    pseq += 1
                while next_ch < NCHK:
                    emit_and(next_ch)
                    next_ch += 1
            # projection quant
            nl = 0
            uost = [0] * 4
            for d in range(2):
                for c in range(NDC):
                    ncols = min(DCH, SH - c * DCH)
                    v.wait_ge(sems["cir"], 16 * (nl + 1))
                    for h in range(2):
                        ob = 2 * (nl % 2) + h
                        pq = tmpq[nl % 2]
                        qs = qsb[nl % 2]
                        v.wait_ge(sems["pmm"], 2 * nl + h + 1)
                        if uost[ob] >= 1:
                            v.wait_ge(s_ost[ob], 16 * uost[ob])
                        ci_b = cirep[nl % 2][:, :ncols]
                        v.scalar_tensor_tensor(
                            pq[:, 1:1 + ncols], pps[ob][:, :ncols], 0.0, ci_b,
                            mybir.AluOpType.bypass, mybir.AluOpType.mult)
                        fcb_b = fcb_sb[:, h:h + 1].to_broadcast([128, ncols])
                        v.scalar_tensor_tensor(
                            pq[:, 1:1 + ncols], pq[:, 1:1 + ncols], 0.0, fcb_b,
                            mybir.AluOpType.bypass, mybir.AluOpType.add)
                        # reduce writes the raw absmax straight into the scale
                        # output (host divides by 127).  The tiny ops that
                        # consume it run on the ACT engine (cross-engine sems
                        # give real ordering; small DVE ops race the reduce).
                        osc_ap = osc_sb[:, (d * NDC + c) * 2 + h:
                                        (d * NDC + c) * 2 + h + 1]
                        v.tensor_reduce(osc_ap, pq[:, 0:1 + ncols],
                                        mybir.AxisListType.X, mybir.AluOpType.max,
                                        apply_absolute_value=True
                                        ).then_inc(sems["qmx"], 1)
                        v.wait_ge(sems["qwd"], 2 * nl + h + 1)
                        v.reciprocal(rcb2[:], rcb[:])
                        inv_b = rcb2[:, 0:1].to_broadcast([128, ncols])
                        v.scalar_tensor_tensor(
                            qs[:, :ncols], pq[:, 1:1 + ncols], 0.0, inv_b,
                            mybir.AluOpType.bypass, mybir.AluOpType.mult)
                        if taps and nl == 0 and h == 0:
                            v.tensor_copy(tap_sb[:, 0:DCH], pps[ob][:, :ncols])
                            v.tensor_copy(tap_sb[:, DCH:DCH + ncols], pq[:, 1:1 + ncols])
                            v.tensor_copy(tap_sb[:, 2 * DCH:2 * DCH + 64], rcb[:])
                            v.tensor_copy(tap_sb[:, 2 * DCH + 64:2 * DCH + 128], rcb2[:])
                            v.tensor_copy(tap_sb[:, 3 * DCH:3 * DCH + ncols], qs[:, :ncols])
                        uost[ob] += 1
                        v.tensor_copy(ostage[ob][:, :ncols], qs[:, :ncols]
                                      ).then_inc(sems["oact"], 1)
                    nl += 1

        # ======== SCALAR: X write DMAs, page->stage copies, hT DMAs
        @block.scalar
        def _(a):
            a.wait_ge(sems["zf"], 16 * ZF_DMAS)
            gu = 0
            for pi, (d, k) in enumerate(PASSES):
                for r in range(R):
                    for c in range(XCALL):
                        a.wait_ge(sems["xst"], gu + 1)
                        tb = XA[d, k][r] if c < 32 else XB[d, k][r]
                        row0 = c * 1024 if c < 32 else (c - 32) * 1024
                        xs = xstage[gu % 4]
                        if c == XCALL - 1:
                            d1 = tb[row0:row0 + 768, 0:64] \
                                .rearrange("(j l) m -> l j m", l=128)
                            a.dma_start(d1, xs[:, 0:6, :]).then_inc(sems["xout"], 16)
                            a.dma_start(tb[row0 + 768:row0 + 848, 0:64],
                                        xs[0:80, 6, :]).then_inc(sems["xout"], 16)
                        else:
                            dd = tb[row0:row0 + 1024, 0:64] \
                                .rearrange("(j l) m -> l j m", l=128)
                            a.dma_start(dd, xs[:]).then_inc(sems["xout"], 16)
                        gu += 1
            # page -> stage -> hT
            for pi, (d, k) in enumerate(PASSES):
                for p in range(NPAGE):
                    st = p // SPS
                    stglob = pi * NSTG + st
                    pglob = pi * NPAGE + p
                    a.wait_ge(sems["pe"], pi * MM_PASS + int(mm_cum[p + 1]))
                    if stglob >= NSTB and p % SPS == 0:
                        a.wait_ge(s_stg[stglob % NSTB], 16 * (stglob // NSTB))
                    a.copy(stage[stglob % NSTB][:, (p % SPS) * 128:(p % SPS + 1) * 128],
                           pages[pglob % NPB]).then_inc(sems["act"], 1)
                    if p % SPS == SPS - 1 or p == NPAGE - 1:
                        p0 = st * SPS
                        npg = p - p0 + 1
                        a.dma_start(hT[d, k][:, p0 * 128:(p0 + npg) * 128],
                                    stage[stglob % NSTB][:, :npg * 128]
                                    ).then_inc(s_stg[stglob % NSTB], 16)
            # quant scale widen: rcb = mx/127 broadcast (ACT per-partition
            # scale port; cross-engine sems order it against the reduce)
            qi = 0
            for d in range(2):
                for c in range(NDC):
                    for h in range(2):
                        osc_ap = osc_sb[:, (d * NDC + c) * 2 + h:
                                        (d * NDC + c) * 2 + h + 1]
                        a.wait_ge(sems["qmx"], qi + 1)
                        if qi >= 1:
                            a.wait_ge(sems["oact"], qi)
                        a.activation(rcb[:], onesq[:],
                                     mybir.ActivationFunctionType.Copy,
                                     scale=osc_ap).then_inc(sems["qwd"], 1)
                        qi += 1

        # ======== SYNC: zero-fills, idx/bnd/proj loads, out DMAs
        @block.sync
        def _(s):
            s.wait_ge(sems["zini"], 1)
            for pi, (d, k) in enumerate(PASSES):
                for r in range(R):
                    for cc in range(16):
                        dst = XB[d, k][r][cc * 2048:(cc + 1) * 2048, :] \
                            .rearrange("(j l) m -> l j m", l=128)
                        s.dma_start(dst, zfill[:]).then_inc(sems["zf"], 16)
            for pi, (d, k) in enumerate(PASSES):
                if pi >= 1:
                    s.wait_ge(sems["xst"], pi * R * XCALL)
                for rep in range(8):
                    s.dma_start(xgsb[rep * 16:(rep + 1) * 16, :], xg_d[d, k][:]
                                ).then_inc(sems["xgl"], 16)
                if k == 0:
                    s.dma_start(cjsb[:], cj_d[d][:]).then_inc(sems["xcj"], 16)
            # edge loads, lazily interleaved like the vector engine
            for pi, (d, k) in enumerate(PASSES):
                next_ch = 0

                def emit_graw(ch, pi=pi, d=d):
                    if pi * NCHK + ch >= 2:
                        s.wait_ge(sems["gab"], pi * NCHK + ch - 1)
                    for rep in range(8):
                        s.dma_start(
                            graw[ch % 2][rep * 16:(rep + 1) * 16, :],
                            gD[d][:, ch * CH * 64:(ch + 1) * CH * 64]
                        ).then_inc(sems["graw"], 16)

                for p in range(NPAGE):
                    r, w0, nwin = geom[p]
                    ch_need = min((w0 + nwin) // (8 * CH) + 1, NCHK)
                    while next_ch < ch_need:
                        emit_graw(next_ch)
                        next_ch += 1
                    pseq = pi * NPAGE + p
                    if pseq >= NSB:
                        s.wait_ge(sems["bcv"], pseq - NSB + 1)
                    s.dma_start(bndi[pseq % NSB][:, 0:257],
                                bndD[d][0:1, p * 257:(p + 1) * 257]
                                .to_broadcast([128, 257])).then_inc(sems["bndl"], 16)
                while next_ch < NCHK:
                    emit_graw(next_ch)
                    next_ch += 1
            # projection
            NSTGALL = len(PASSES) * NSTG
            for b in range(NSTB):
                occ = (NSTGALL - b + NSTB - 1) // NSTB
                s.wait_ge(s_stg[b], 16 * occ)
            nl = 0
            uost = [0] * 4
            for d in range(2):
                for c in range(NDC):
                    ncols = min(DCH, SH - c * DCH)
                    if nl >= 2:
                        s.wait_ge(sems["pmm"], 2 * (nl - 1))
                        # cirep/prhs of nl-2 fully consumed by quant chains
                        s.wait_ge(sems["oact"], 2 * (nl - 1))
                    for rk in range(R * NK):
                        r, kk = rk // NK, rk % NK
                        s.dma_start(
                            prhs[nl % 2][:, rk, :ncols],
                            hT[d, kk][:, r * PPR * 128 + c * DCH:
                                      r * PPR * 128 + c * DCH + ncols]
                        ).then_inc(s_pin[nl % 2], 16)
                    s.dma_start(cirep[nl % 2][:, :ncols],
                                ci_d[d][0:1, c * DCH:c * DCH + ncols]
                                .to_broadcast([128, ncols])).then_inc(sems["cir"], 16)
                    for h in range(2):
                        ob = 2 * (nl % 2) + h
                        s.wait_ge(sems["oact"], 2 * nl + h + 1)
                        s.dma_start(outT_d[d, h * 128:(h + 1) * 128,
                                           c * DCH:c * DCH + ncols],
                                    ostage[ob][:, :ncols]).then_inc(s_ost[ob], 16)
                        uost[ob] += 1
                    nl += 1
            s.wait_ge(sems["oact"], 4 * NDC)
            s.dma_start(osc_d[:], osc_sb[:]).then_inc(sems["osc"], 16)
            if taps:
                s.dma_start(tap_d[:], tap_sb[:]).then_inc(sems["osc"], 16)
            for ob in range(4):
                s.wait_ge(s_ost[ob], 16 * uost[ob])
            s.wait_ge(sems["osc"], 32 if taps else 16)

    # X write DMA cumulative count helper used by vector (defined late but
    # bound early via closure — compute as plain function of gu)
    nc.compile()
    return nc


def xout_cum(n_calls):
    """X write DMA sem count once the first n_calls X calls are stored.
    Each r-block is 49 calls -> 50 DMAs (last call is split in two)."""
    full, rem = divmod(n_calls, 49)
    return 16 * (full * 50 + rem)


# ======================================================================
from concourse.bass_utils import run_bass_kernel_spmd as _run_spmd

_CACHE = {}


def kernel(**inputs):
    """GCMC layer on 8 trn2 NeuronCores. Returns (drug_out, dis_out) f32."""
    cfg = Cfg(50000, 1024, 8, wpp=12)
    maps = build_inputs(cfg, inputs)
    if "nc" not in _CACHE:
        _CACHE["nc"] = build_kernel(cfg)
    res = _run_spmd(_CACHE["nc"], maps, list(range(cfg.NC)))
    return assemble_output(cfg, res.results)
